# revision 3
# baseline (speedup 1.0000x reference)
"""Expert-parallel SwiGLU MLP (MoE experts) for 8 Trainium2 NeuronCores.

Problem: routed_in_egD [E*G, D] fp32, w1/w3 [E, D, F], w2 [E, F, D], E=8,
G=2048, D=2048, F=5632.  reference:
    x_egD = routed.reshape(E, G, D)
    mid   = silu(x @ w1) * (x @ w3)          # [E, G, F]
    out   = (mid @ w2).reshape(E*G, D)

Sharding: expert-parallel — core e gets expert e's x slice + weights; no
collectives.  Each core runs three 2048x2048x5632-class GEMMs (~142 GFLOP).

Per-core kernel (all matmuls fp32r unless noted):
  phase 0: PE-transpose x [G,D] -> xT [D,G] resident in SBUF (fp32r).
  phase 1: for each f-chunk (128 rows of F): gateT/upT = w1/w3-chunk.T @ x
           accumulated over D in PSUM; SwiGLU on ACT+DVE; midT [F,G] spilled
           to DRAM as bf16.
  phase 2: out[g,d] = sum_f midT[f,g]*w2[f,d]: mid chunks stationary (bf16),
           w2 streamed (DMA-cast fp32->bf16), PSUM accumulation over F.
           Output written in natural [G, D] layout.
"""

import numpy as np

import concourse.mybir as mybir
import concourse.tile as tile
from concourse import bacc
from concourse.bass_utils import run_bass_kernel_spmd
from concourse.masks import make_identity

E, G, D, F = 8, 2048, 2048, 5632
P = 128
DO = D // P      # 16 d-chunks
FC = F // P      # 44 f-chunks
GO = G // P      # 16 g-chunks

F32 = mybir.dt.float32
F32R = mybir.dt.float32r
BF16 = mybir.dt.bfloat16


def build_nc():
    nc = bacc.Bacc("TRN2", target_bir_lowering=False)
    x = nc.dram_tensor("x", [G, D], F32, kind="ExternalInput").ap()
    w1 = nc.dram_tensor("w1", [D, F], F32, kind="ExternalInput").ap()
    w2 = nc.dram_tensor("w2", [F, D], F32, kind="ExternalInput").ap()
    w3 = nc.dram_tensor("w3", [D, F], F32, kind="ExternalInput").ap()
    out = nc.dram_tensor("out", [G, D], F32, kind="ExternalOutput").ap()

    w1r = w1.rearrange("(do p) f -> p do f", p=P)
    w3r = w3.rearrange("(do p) f -> p do f", p=P)
    w2r = w2.rearrange("(fo p) d -> p fo d", p=P)

    with tile.TileContext(nc) as tc:
        with tc.tile_pool(name="dram", bufs=1, space="DRAM") as dram:
            mid = dram.tile([F, G], BF16)
            mid_r = mid.rearrange("(fo p) g -> p fo g", p=P)

            with tc.tile_pool(name="xtp", bufs=1) as xtp:
                xT = xtp.tile([P, DO, G], F32R)

                # ---- phase 0: x [G, D] -> xT [d_in, d_out, g] (fp32r)
                with (
                    tc.tile_pool(name="p0", bufs=2) as p0,
                    tc.tile_pool(name="idp", bufs=1) as idp,
                    tc.tile_pool(name="p0ps", bufs=4, space="PSUM") as p0ps,
                ):
                    ident = idp.tile([P, P], F32)
                    make_identity(nc, ident)
                    for go in range(GO):
                        xs = p0.tile([P, D], F32, tag="xs")
                        nc.sync.dma_start(xs, x[go * P : (go + 1) * P, :])
                        for d4 in range(DO // 4):
                            tp = p0ps.tile([P, 4, P], F32, tag="tp")
                            for j in range(4):
                                do = d4 * 4 + j
                                nc.tensor.transpose(
                                    tp[:, j], xs[:, do * P : (do + 1) * P], ident
                                )
                            nc.vector.tensor_copy(
                                xT[:, d4 * 4 : (d4 + 1) * 4, go * P : (go + 1) * P],
                                tp,
                            )

                # ---- phase 1: midT[f, g] = silu(w1.T x) * (w3.T x), spill bf16
                with (
                    tc.tile_pool(name="wp", bufs=2) as wp,
                    tc.tile_pool(name="sp", bufs=2) as sp,
                    tc.tile_pool(name="mp", bufs=3) as mp,
                    tc.tile_pool(name="ps1", bufs=2, space="PSUM") as ps1,
                ):
                    for fc in range(FC):
                        w1t = wp.tile([P, DO, P], F32R, tag="w1")
                        nc.gpsimd.dma_start(w1t, w1r[:, :, fc * P : (fc + 1) * P])
                        w3t = wp.tile([P, DO, P], F32R, tag="w3")
                        nc.gpsimd.dma_start(w3t, w3r[:, :, fc * P : (fc + 1) * P])
                        for gh in range(2):
                            pg = ps1.tile([P, 2, 512], F32, tag="pg")
                            pu = ps1.tile([P, 2, 512], F32, tag="pu")
                            for d in range(DO):
                                st, sp_ = (d == 0), (d == DO - 1)
                                for j in range(2):
                                    gsl = slice((gh * 2 + j) * 512, (gh * 2 + j + 1) * 512)
                                    nc.tensor.matmul(
                                        pg[:, j], w1t[:, d], xT[:, d, gsl],
                                        start=st, stop=sp_,
                                    )
                                    nc.tensor.matmul(
                                        pu[:, j], w3t[:, d], xT[:, d, gsl],
                                        start=st, stop=sp_,
                                    )
                            tmp = sp.tile([P, 2, 512], F32, tag="tmp")
                            nc.scalar.activation(
                                tmp, pg, mybir.ActivationFunctionType.Silu
                            )
                            mo = mp.tile([P, 2, 512], BF16, tag="mo")
                            nc.vector.tensor_mul(mo, tmp, pu)
                            nc.sync.dma_start(
                                mid[fc * P : (fc + 1) * P, gh * 1024 : (gh + 1) * 1024],
                                mo,
                            )

            # ---- phase 2: out[g, d] = midT.T @ w2 (bf16 x bf16, fp32 psum)
            with (
                tc.tile_pool(name="w2p", bufs=2) as w2p,
                tc.tile_pool(name="mqp", bufs=3) as mqp,
                tc.tile_pool(name="op", bufs=4) as op,
                tc.tile_pool(name="ps2", bufs=4, space="PSUM") as ps2,
            ):
                for dq in range(4):
                    w2q = w2p.tile([P, FC, 512], BF16, tag="w2q")
                    nc.gpsimd.dma_start(w2q, w2r[:, :, dq * 512 : (dq + 1) * 512])
                    for gp in range(8):
                        mq = mqp.tile([P, FC, 256], BF16, tag="mq")
                        nc.sync.dma_start(mq, mid_r[:, :, gp * 256 : (gp + 1) * 256])
                        po0 = ps2.tile([P, 512], F32, tag="po0")
                        po1 = ps2.tile([P, 512], F32, tag="po1")
                        for fo in range(FC):
                            st, sp_ = (fo == 0), (fo == FC - 1)
                            nc.tensor.matmul(
                                po0, mq[:, fo, 0:128], w2q[:, fo], start=st, stop=sp_
                            )
                            nc.tensor.matmul(
                                po1, mq[:, fo, 128:256], w2q[:, fo], start=st, stop=sp_
                            )
                        for gc2, po in ((0, po0), (1, po1)):
                            ot = op.tile([P, 512], F32, tag="ot")
                            nc.vector.tensor_copy(ot, po)
                            g0 = (gp * 2 + gc2) * P
                            nc.sync.dma_start(
                                out[g0 : g0 + P, dq * 512 : (dq + 1) * 512], ot
                            )
    nc.compile()
    return nc


_NC_CACHE = None


def _get_nc():
    global _NC_CACHE
    if _NC_CACHE is None:
        _NC_CACHE = build_nc()
    return _NC_CACHE


def _in_maps(routed_in_egD, w1, w2, w3):
    x = np.ascontiguousarray(np.asarray(routed_in_egD, dtype=np.float32))
    w1 = np.ascontiguousarray(np.asarray(w1, dtype=np.float32))
    w2 = np.ascontiguousarray(np.asarray(w2, dtype=np.float32))
    w3 = np.ascontiguousarray(np.asarray(w3, dtype=np.float32))
    x_e = x.reshape(E, G, D)
    return [
        {"x": x_e[e], "w1": w1[e], "w2": w2[e], "w3": w3[e]} for e in range(E)
    ]


def kernel(routed_in_egD, w1, w2, w3):
    nc = _get_nc()
    res = run_bass_kernel_spmd(
        nc, _in_maps(routed_in_egD, w1, w2, w3), core_ids=list(range(E))
    )
    return np.concatenate([r["out"] for r in res.results], axis=0)


def run_traced(routed_in_egD, w1, w2, w3, **trace_kwargs):
    """For test.py: run with NTFF tracing; returns (full_out, BassKernelResults)."""
    nc = _get_nc()
    res = run_bass_kernel_spmd(
        nc,
        _in_maps(routed_in_egD, w1, w2, w3),
        core_ids=list(range(E)),
        trace=True,
        **trace_kwargs,
    )
    out = np.concatenate([r["out"] for r in res.results], axis=0)
    return out, res


# revision 5
# speedup vs baseline: 1.0122x; 1.0122x over previous
"""Expert-parallel SwiGLU MLP (MoE experts) for 8 Trainium2 NeuronCores.

Problem: routed_in_egD [E*G, D] fp32, w1/w3 [E, D, F], w2 [E, F, D], E=8,
G=2048, D=2048, F=5632.  reference:
    x_egD = routed.reshape(E, G, D)
    mid   = silu(x @ w1) * (x @ w3)          # [E, G, F]
    out   = (mid @ w2).reshape(E*G, D)

Sharding: expert-parallel — core e gets expert e's x slice + weights; no
collectives.  Each core runs three 2048x2048x5632-class GEMMs (~142 GFLOP).

Per-core kernel (matmuls fp32r in stage 1, bf16 in stage 2, fp32 PSUM):
  phase 0: PE-transpose x [G,D] -> xT [D,G] in SBUF (fp32r), split in two
           g-half tiles so the first half's SBUF can be released early.
  phase 1: per g-half, per f-chunk: gateT/upT = w1/w3-chunk.T @ x accumulated
           over D in PSUM; SwiGLU (ACT silu + DVE mul); midT [F,G] spilled to
           DRAM as bf16.  After the first g-half completes, its xT SBUF is
           released and phase 2's first w2 panel prefetches in its place.
  phase 2: out[g,d] = sum_f midT[f,g]*w2[f,d]: mid chunks stationary (bf16),
           w2 panels DMA-cast fp32->bf16, PSUM accumulation over F.  Output
           lands in natural [G, D] layout.
"""

import numpy as np

import concourse.mybir as mybir
import concourse.tile as tile
from concourse import bacc
from concourse.bass_utils import run_bass_kernel_spmd
from concourse.masks import make_identity

E, G, D, F = 8, 2048, 2048, 5632
P = 128
DO = D // P      # 16 d-chunks
FC = F // P      # 44 f-chunks
GO = G // P      # 16 g-chunks
GHW = G // 2     # 1024, g-half width

F32 = mybir.dt.float32
F32R = mybir.dt.float32r
BF16 = mybir.dt.bfloat16


def build_nc():
    nc = bacc.Bacc("TRN2", target_bir_lowering=False)
    x = nc.dram_tensor("x", [G, D], F32, kind="ExternalInput").ap()
    w1 = nc.dram_tensor("w1", [D, F], F32, kind="ExternalInput").ap()
    w2 = nc.dram_tensor("w2", [F, D], F32, kind="ExternalInput").ap()
    w3 = nc.dram_tensor("w3", [D, F], F32, kind="ExternalInput").ap()
    out = nc.dram_tensor("out", [G, D], F32, kind="ExternalOutput").ap()

    w1r = w1.rearrange("(do p) f -> p do f", p=P)
    w3r = w3.rearrange("(do p) f -> p do f", p=P)
    w2r = w2.rearrange("(fo p) d -> p fo d", p=P)

    with tile.TileContext(nc) as tc:
        dram = tc.alloc_tile_pool(name="dram", bufs=1, space="DRAM")
        mid = dram.tile([F, G], BF16)
        mid_r = mid.rearrange("(fo p) g -> p fo g", p=P)

        # long-lived phase-1 pools (released before phase 2)
        wp = tc.alloc_tile_pool(name="wp", bufs=2)
        sp = tc.alloc_tile_pool(name="sp", bufs=2)
        mp = tc.alloc_tile_pool(name="mp", bufs=3)

        xtp_b = tc.alloc_tile_pool(name="xtpb", bufs=1)
        xT_b = xtp_b.tile([P, DO, GHW], F32R)
        xtp_a = tc.alloc_tile_pool(name="xtpa", bufs=1)
        xT_a = xtp_a.tile([P, DO, GHW], F32R)

        # ---- phase 0: x [G, D] -> xT halves [d_in, d_out, g] (fp32r)
        p0 = tc.alloc_tile_pool(name="p0", bufs=6)
        idp = tc.alloc_tile_pool(name="idp", bufs=1)
        p0ps = tc.alloc_tile_pool(name="p0ps", bufs=4, space="PSUM")
        ident = idp.tile([P, P], F32)
        make_identity(nc, ident)
        for go in range(GO):
            xTh = xT_a if go < GO // 2 else xT_b
            gcol = (go % (GO // 2)) * P
            for q in range(4):
                xsq = p0.tile([P, 512], F32, tag="xs")
                nc.sync.dma_start(
                    xsq, x[go * P : (go + 1) * P, q * 512 : (q + 1) * 512]
                )
                tp = p0ps.tile([P, 4, P], F32, tag="tp")
                for j in range(4):
                    nc.tensor.transpose(tp[:, j], xsq[:, j * P : (j + 1) * P], ident)
                nc.vector.tensor_copy(
                    xTh[:, q * 4 : (q + 1) * 4, gcol : gcol + P], tp
                )
        idp.release()
        p0.release()
        p0ps.release()

        # ---- phase 1: midT[f, g] = silu(w1.T x) * (w3.T x), spill bf16
        ps1 = tc.alloc_tile_pool(name="ps1", bufs=2, space="PSUM")
        w2pre = None
        w2q0 = None
        for gh, xTh in ((0, xT_a), (1, xT_b)):
            for fc in range(FC):
                w1t = wp.tile([P, DO, P], F32R, tag="w1")
                nc.gpsimd.dma_start(w1t, w1r[:, :, fc * P : (fc + 1) * P])
                w3t = wp.tile([P, DO, P], F32R, tag="w3")
                nc.gpsimd.dma_start(w3t, w3r[:, :, fc * P : (fc + 1) * P])
                pg = ps1.tile([P, 2, 512], F32, tag="pg")
                pu = ps1.tile([P, 2, 512], F32, tag="pu")
                for d in range(DO):
                    st, sp_ = (d == 0), (d == DO - 1)
                    for j in range(2):
                        gsl = slice(j * 512, (j + 1) * 512)
                        nc.tensor.matmul(
                            pg[:, j], w1t[:, d], xTh[:, d, gsl], start=st, stop=sp_
                        )
                        nc.tensor.matmul(
                            pu[:, j], w3t[:, d], xTh[:, d, gsl], start=st, stop=sp_
                        )
                tmp = sp.tile([P, 2, 512], F32, tag="tmp")
                nc.scalar.activation(tmp, pg, mybir.ActivationFunctionType.Silu)
                mo = mp.tile([P, 2, 512], BF16, tag="mo")
                nc.vector.tensor_mul(mo, tmp, pu)
                nc.sync.dma_start(
                    mid[fc * P : (fc + 1) * P, gh * GHW : (gh + 1) * GHW], mo
                )
            if gh == 0:
                # free first-half xT; prefetch phase-2's first w2 panel there
                xtp_a.release()
                w2pre = tc.alloc_tile_pool(name="w2pre", bufs=1, side="right")
                w2q0 = w2pre.tile([P, FC, 512], BF16)
                nc.gpsimd.dma_start(w2q0, w2r[:, :, 0:512])
        xtp_b.release()
        mp.release()
        sp.release()
        wp.release()
        ps1.release()

        # ---- phase 2: out[g, d] = midT.T @ w2 (bf16 x bf16, fp32 psum)
        w2p = tc.alloc_tile_pool(name="w2p", bufs=2, side="right")
        mqp = tc.alloc_tile_pool(name="mqp", bufs=2, side="right")
        op = tc.alloc_tile_pool(name="op", bufs=4, side="right")
        ps2 = tc.alloc_tile_pool(name="ps2", bufs=4, space="PSUM")
        for dq in range(4):
            if dq == 0:
                w2q = w2q0
            else:
                w2q = w2p.tile([P, FC, 512], BF16, tag="w2q")
                nc.gpsimd.dma_start(w2q, w2r[:, :, dq * 512 : (dq + 1) * 512])
            for gp in range(8):
                mq = mqp.tile([P, FC, 256], BF16, tag="mq")
                nc.sync.dma_start(mq, mid_r[:, :, gp * 256 : (gp + 1) * 256])
                po0 = ps2.tile([P, 512], F32, tag="po0")
                po1 = ps2.tile([P, 512], F32, tag="po1")
                for fo in range(FC):
                    st, sp_ = (fo == 0), (fo == FC - 1)
                    nc.tensor.matmul(
                        po0, mq[:, fo, 0:128], w2q[:, fo], start=st, stop=sp_
                    )
                    nc.tensor.matmul(
                        po1, mq[:, fo, 128:256], w2q[:, fo], start=st, stop=sp_
                    )
                for gc2, po in ((0, po0), (1, po1)):
                    ot = op.tile([P, 512], F32, tag="ot")
                    nc.vector.tensor_copy(ot, po)
                    g0 = (gp * 2 + gc2) * P
                    nc.sync.dma_start(
                        out[g0 : g0 + P, dq * 512 : (dq + 1) * 512], ot
                    )
        op.release()
        mqp.release()
        w2p.release()
        w2pre.release()
        ps2.release()
        dram.release()
    nc.compile()
    return nc


_NC_CACHE = None


def _get_nc():
    global _NC_CACHE
    if _NC_CACHE is None:
        _NC_CACHE = build_nc()
    return _NC_CACHE


def _in_maps(routed_in_egD, w1, w2, w3):
    x = np.ascontiguousarray(np.asarray(routed_in_egD, dtype=np.float32))
    w1 = np.ascontiguousarray(np.asarray(w1, dtype=np.float32))
    w2 = np.ascontiguousarray(np.asarray(w2, dtype=np.float32))
    w3 = np.ascontiguousarray(np.asarray(w3, dtype=np.float32))
    x_e = x.reshape(E, G, D)
    return [
        {"x": x_e[e], "w1": w1[e], "w2": w2[e], "w3": w3[e]} for e in range(E)
    ]


def kernel(routed_in_egD, w1, w2, w3):
    nc = _get_nc()
    res = run_bass_kernel_spmd(
        nc, _in_maps(routed_in_egD, w1, w2, w3), core_ids=list(range(E))
    )
    return np.concatenate([r["out"] for r in res.results], axis=0)


def run_traced(routed_in_egD, w1, w2, w3, **trace_kwargs):
    """For test.py: run with NTFF tracing; returns (full_out, BassKernelResults)."""
    nc = _get_nc()
    res = run_bass_kernel_spmd(
        nc,
        _in_maps(routed_in_egD, w1, w2, w3),
        core_ids=list(range(E)),
        trace=True,
        **trace_kwargs,
    )
    out = np.concatenate([r["out"] for r in res.results], axis=0)
    return out, res


# revision 11
# speedup vs baseline: 1.0223x; 1.0100x over previous
"""Expert-parallel SwiGLU MLP (MoE experts) for 8 Trainium2 NeuronCores.

Problem: routed_in_egD [E*G, D] fp32, w1/w3 [E, D, F], w2 [E, F, D], E=8,
G=2048, D=2048, F=5632.  reference:
    x_egD = routed.reshape(E, G, D)
    mid   = silu(x @ w1) * (x @ w3)          # [E, G, F]
    out   = (mid @ w2).reshape(E*G, D)

Sharding: expert-parallel — core e gets expert e's x slice + weights; no
collectives.  Each core runs three 2048x2048x5632-class GEMMs (~142 GFLOP).

Per-core kernel (matmuls fp32r in stage 1, bf16 in stage 2, fp32 PSUM):
  phase 0: PE-transpose x [G,D] -> xT [D,G] in SBUF (fp32r), split in two
           g-half tiles so the first half's SBUF can be released early.
  phase 1: per g-half, per f-chunk: gateT/upT = w1/w3-chunk.T @ x accumulated
           over D in PSUM; SwiGLU (ACT silu + DVE mul); midT [F,G] spilled to
           DRAM as bf16.  After the first g-half completes, its xT SBUF is
           released and phase 2's first w2 panel prefetches in its place.
  phase 2: out[g,d] = sum_f midT[f,g]*w2[f,d]: mid chunks stationary (bf16),
           w2 panels DMA-cast fp32->bf16, PSUM accumulation over F.  Output
           lands in natural [G, D] layout.
"""

import numpy as np

import concourse.mybir as mybir
import concourse.tile as tile
from concourse import bacc
from concourse.bass_utils import run_bass_kernel_spmd
from concourse.masks import make_identity

E, G, D, F = 8, 2048, 2048, 5632
P = 128
DO = D // P      # 16 d-chunks
FC = F // P      # 44 f-chunks
GO = G // P      # 16 g-chunks
GHW = G // 2     # 1024, g-half width

F32 = mybir.dt.float32
F32R = mybir.dt.float32r
BF16 = mybir.dt.bfloat16


def build_nc():
    nc = bacc.Bacc("TRN2", target_bir_lowering=False)
    x = nc.dram_tensor("x", [G, D], F32, kind="ExternalInput").ap()
    w1 = nc.dram_tensor("w1", [D, F], F32, kind="ExternalInput").ap()
    w2 = nc.dram_tensor("w2", [F, D], F32, kind="ExternalInput").ap()
    w3 = nc.dram_tensor("w3", [D, F], F32, kind="ExternalInput").ap()
    out = nc.dram_tensor("out", [G, D], F32, kind="ExternalOutput").ap()

    w1r = w1.rearrange("(do p) f -> p do f", p=P)
    w3r = w3.rearrange("(do p) f -> p do f", p=P)
    w2r = w2.rearrange("(fo p) d -> p fo d", p=P)

    with tile.TileContext(nc) as tc:
        dram = tc.alloc_tile_pool(name="dram", bufs=1, space="DRAM")
        mid = dram.tile([F, G], BF16)
        mid_r = mid.rearrange("(fo p) g -> p fo g", p=P)

        # long-lived phase-1 pools (released before phase 2)
        wp = tc.alloc_tile_pool(name="wp", bufs=2)
        mp = tc.alloc_tile_pool(name="mp", bufs=3)

        xtp_b = tc.alloc_tile_pool(name="xtpb", bufs=1)
        xT_b = xtp_b.tile([P, DO, GHW], F32R)
        xtp_a = tc.alloc_tile_pool(name="xtpa", bufs=1)
        xT_a = xtp_a.tile([P, DO, GHW], F32R)

        # ---- phase 0: x [G, D] -> xT halves [d_in, d_out, g] (fp32r)
        p0 = tc.alloc_tile_pool(name="p0", bufs=6)
        idp = tc.alloc_tile_pool(name="idp", bufs=1)
        p0ps = tc.alloc_tile_pool(name="p0ps", bufs=4, space="PSUM")
        ident = idp.tile([P, P], F32)
        make_identity(nc, ident)
        for go in range(GO):
            xTh = xT_a if go < GO // 2 else xT_b
            gcol = (go % (GO // 2)) * P
            for q in range(4):
                xsq = p0.tile([P, 512], F32, tag="xs")
                nc.sync.dma_start(
                    xsq, x[go * P : (go + 1) * P, q * 512 : (q + 1) * 512]
                )
                tp = p0ps.tile([P, 4, P], F32, tag="tp")
                for j in range(4):
                    nc.tensor.transpose(tp[:, j], xsq[:, j * P : (j + 1) * P], ident)
                nc.vector.tensor_copy(
                    xTh[:, q * 4 : (q + 1) * 4, gcol : gcol + P], tp
                )
        idp.release()
        p0.release()
        p0ps.release()

        # ---- phase 1: midT[f, g] = silu(w1.T x) * (w3.T x), spill bf16
        ps1 = tc.alloc_tile_pool(name="ps1", bufs=2, space="PSUM")
        w2pre = None
        w2q0 = None
        mq0 = None
        mqp = None
        # w2 panel-0 prefetch, split in chunks interleaved across the second
        # g-half so the single SWDGE queue never starves the w1/w3 stream.
        w2chunks = {4: 0, 14: 1, 24: 2, 34: 3}
        for gh, xTh in ((0, xT_a), (1, xT_b)):
            for fc in range(FC):
                w1t = wp.tile([P, DO, P], F32R, tag="w1")
                nc.gpsimd.dma_start(w1t, w1r[:, :, fc * P : (fc + 1) * P])
                w3t = wp.tile([P, DO, P], F32R, tag="w3")
                nc.gpsimd.dma_start(w3t, w3r[:, :, fc * P : (fc + 1) * P])
                if gh == 1 and fc in w2chunks:
                    k = w2chunks[fc]
                    nc.gpsimd.dma_start(
                        w2q0[:, k * 11 : (k + 1) * 11, :],
                        w2r[:, k * 11 : (k + 1) * 11, 0:512],
                    )
                pg = ps1.tile([P, 2, 512], F32, tag="pg")
                pu = ps1.tile([P, 2, 512], F32, tag="pu")
                for d in range(DO):
                    st, sp_ = (d == 0), (d == DO - 1)
                    for j in range(2):
                        gsl = slice(j * 512, (j + 1) * 512)
                        nc.tensor.matmul(
                            pg[:, j], w1t[:, d], xTh[:, d, gsl], start=st, stop=sp_
                        )
                        nc.tensor.matmul(
                            pu[:, j], w3t[:, d], xTh[:, d, gsl], start=st, stop=sp_
                        )
                mo = mp.tile([P, 2, 512], BF16, tag="mo")
                nc.scalar.activation(mo, pg, mybir.ActivationFunctionType.Silu)
                nc.vector.tensor_mul(mo, mo, pu)
                nc.sync.dma_start(
                    mid[fc * P : (fc + 1) * P, gh * GHW : (gh + 1) * GHW], mo
                )
                if gh == 1 and fc == 0:
                    # mid cols 0:1024 are complete (gh0); prefetch phase-2's
                    # first mid panel on the HWDGE queue during gh1 compute
                    mq0 = mqp.tile([P, FC, 256], BF16, tag="mq")
                    nc.sync.dma_start(mq0, mid_r[:, :, 0:256])
            if gh == 0:
                # free first-half xT; phase-2 prefetch pools take its place
                xtp_a.release()
                w2pre = tc.alloc_tile_pool(name="w2pre", bufs=1, side="right")
                w2q0 = w2pre.tile([P, FC, 512], BF16)
                mqp = tc.alloc_tile_pool(name="mqp", bufs=2, side="right")
        xtp_b.release()
        mp.release()
        wp.release()
        ps1.release()

        # ---- phase 2: out[g, d] = midT.T @ w2 (bf16 x bf16, fp32 psum)
        w2p = tc.alloc_tile_pool(name="w2p", bufs=2, side="right")
        op = tc.alloc_tile_pool(name="op", bufs=3, side="right")
        ps2 = tc.alloc_tile_pool(name="ps2", bufs=4, space="PSUM")
        for dq in range(4):
            if dq == 0:
                w2q = w2q0
            else:
                w2q = w2p.tile([P, FC, 512], BF16, tag="w2q")
                nc.gpsimd.dma_start(w2q, w2r[:, :, dq * 512 : (dq + 1) * 512])
            for gp in range(8):
                if dq == 0 and gp == 0:
                    mq = mq0
                else:
                    mq = mqp.tile([P, FC, 256], BF16, tag="mq")
                    nc.sync.dma_start(mq, mid_r[:, :, gp * 256 : (gp + 1) * 256])
                po0 = ps2.tile([P, 512], F32, tag="po0")
                po1 = ps2.tile([P, 512], F32, tag="po1")
                for fo in range(FC):
                    st, sp_ = (fo == 0), (fo == FC - 1)
                    nc.tensor.matmul(
                        po0, mq[:, fo, 0:128], w2q[:, fo], start=st, stop=sp_
                    )
                    nc.tensor.matmul(
                        po1, mq[:, fo, 128:256], w2q[:, fo], start=st, stop=sp_
                    )
                for gc2, po in ((0, po0), (1, po1)):
                    ot = op.tile([P, 512], F32, tag="ot")
                    nc.vector.tensor_copy(ot, po)
                    g0 = (gp * 2 + gc2) * P
                    nc.sync.dma_start(
                        out[g0 : g0 + P, dq * 512 : (dq + 1) * 512], ot
                    )
        op.release()
        w2p.release()
        mqp.release()
        w2pre.release()
        ps2.release()
        dram.release()
    nc.compile()
    return nc


_NC_CACHE = None


def _get_nc():
    global _NC_CACHE
    if _NC_CACHE is None:
        _NC_CACHE = build_nc()
    return _NC_CACHE


def _in_maps(routed_in_egD, w1, w2, w3):
    x = np.ascontiguousarray(np.asarray(routed_in_egD, dtype=np.float32))
    w1 = np.ascontiguousarray(np.asarray(w1, dtype=np.float32))
    w2 = np.ascontiguousarray(np.asarray(w2, dtype=np.float32))
    w3 = np.ascontiguousarray(np.asarray(w3, dtype=np.float32))
    x_e = x.reshape(E, G, D)
    return [
        {"x": x_e[e], "w1": w1[e], "w2": w2[e], "w3": w3[e]} for e in range(E)
    ]


def kernel(routed_in_egD, w1, w2, w3):
    nc = _get_nc()
    res = run_bass_kernel_spmd(
        nc, _in_maps(routed_in_egD, w1, w2, w3), core_ids=list(range(E))
    )
    return np.concatenate([r["out"] for r in res.results], axis=0)


def run_traced(routed_in_egD, w1, w2, w3, **trace_kwargs):
    """For test.py: run with NTFF tracing; returns (full_out, BassKernelResults)."""
    nc = _get_nc()
    res = run_bass_kernel_spmd(
        nc,
        _in_maps(routed_in_egD, w1, w2, w3),
        core_ids=list(range(E)),
        trace=True,
        **trace_kwargs,
    )
    out = np.concatenate([r["out"] for r in res.results], axis=0)
    return out, res


# revision 12
# speedup vs baseline: 1.0291x; 1.0066x over previous
"""Expert-parallel SwiGLU MLP (MoE experts) for 8 Trainium2 NeuronCores.

Problem: routed_in_egD [E*G, D] fp32, w1/w3 [E, D, F], w2 [E, F, D], E=8,
G=2048, D=2048, F=5632.  reference:
    x_egD = routed.reshape(E, G, D)
    mid   = silu(x @ w1) * (x @ w3)          # [E, G, F]
    out   = (mid @ w2).reshape(E*G, D)

Sharding: expert-parallel — core e gets expert e's x slice + weights; no
collectives.  Each core runs three 2048x2048x5632-class GEMMs (~142 GFLOP).

Per-core kernel (matmuls fp32r in stage 1, bf16 in stage 2, fp32 PSUM):
  phase 0: PE-transpose x [G,D] -> xT [D,G] in SBUF (fp32r), split in two
           g-half tiles so the first half's SBUF can be released early.
  phase 1: per g-half, per f-chunk: gateT/upT = w1/w3-chunk.T @ x accumulated
           over D in PSUM; SwiGLU (ACT silu + DVE mul); midT [F,G] spilled to
           DRAM as bf16.  After the first g-half completes, its xT SBUF is
           released and phase 2's first w2 panel prefetches in its place.
  phase 2: out[g,d] = sum_f midT[f,g]*w2[f,d]: mid chunks stationary (bf16),
           w2 panels DMA-cast fp32->bf16, PSUM accumulation over F.  Output
           lands in natural [G, D] layout.
"""

import numpy as np

import concourse.mybir as mybir
import concourse.tile as tile
from concourse import bacc
from concourse.bass_utils import run_bass_kernel_spmd
from concourse.masks import make_identity

E, G, D, F = 8, 2048, 2048, 5632
P = 128
DO = D // P      # 16 d-chunks
FC = F // P      # 44 f-chunks
GO = G // P      # 16 g-chunks
GHW = G // 2     # 1024, g-half width

F32 = mybir.dt.float32
F32R = mybir.dt.float32r
BF16 = mybir.dt.bfloat16


def build_nc():
    nc = bacc.Bacc("TRN2", target_bir_lowering=False)
    x = nc.dram_tensor("x", [G, D], F32, kind="ExternalInput").ap()
    w1 = nc.dram_tensor("w1", [D, F], F32, kind="ExternalInput").ap()
    w2 = nc.dram_tensor("w2", [F, D], F32, kind="ExternalInput").ap()
    w3 = nc.dram_tensor("w3", [D, F], F32, kind="ExternalInput").ap()
    out = nc.dram_tensor("out", [G, D], F32, kind="ExternalOutput").ap()

    w1r = w1.rearrange("(do p) f -> p do f", p=P)
    w3r = w3.rearrange("(do p) f -> p do f", p=P)
    w2r = w2.rearrange("(fo p) d -> p fo d", p=P)

    with tile.TileContext(nc) as tc:
        dram = tc.alloc_tile_pool(name="dram", bufs=1, space="DRAM")
        mid = dram.tile([F, G], BF16)
        mid_r = mid.rearrange("(fo p) g -> p fo g", p=P)

        # long-lived phase-1 pools (released before phase 2)
        wp = tc.alloc_tile_pool(name="wp", bufs=2)
        mp = tc.alloc_tile_pool(name="mp", bufs=3)

        xtp_b = tc.alloc_tile_pool(name="xtpb", bufs=1)
        xT_b = xtp_b.tile([P, DO, GHW], F32R)
        xtp_a = tc.alloc_tile_pool(name="xtpa", bufs=1)
        xT_a = xtp_a.tile([P, DO, GHW], F32R)

        # ---- phase 0: x [G, D] -> xT halves [d_in, d_out, g] (fp32r)
        p0 = tc.alloc_tile_pool(name="p0", bufs=8)
        idp = tc.alloc_tile_pool(name="idp", bufs=1)
        p0ps = tc.alloc_tile_pool(name="p0ps", bufs=6, space="PSUM")
        ident = idp.tile([P, P], F32)
        make_identity(nc, ident)
        for go in range(GO):
            xTh = xT_a if go < GO // 2 else xT_b
            gcol = (go % (GO // 2)) * P
            for q in range(4):
                xsq = p0.tile([P, 512], F32, tag="xs")
                nc.sync.dma_start(
                    xsq, x[go * P : (go + 1) * P, q * 512 : (q + 1) * 512]
                )
                tp = p0ps.tile([P, 4, P], F32, tag="tp")
                for j in range(4):
                    nc.tensor.transpose(tp[:, j], xsq[:, j * P : (j + 1) * P], ident)
                nc.vector.tensor_copy(
                    xTh[:, q * 4 : (q + 1) * 4, gcol : gcol + P], tp
                )
        idp.release()
        p0.release()
        p0ps.release()

        # ---- phase 1: midT[f, g] = silu(w1.T x) * (w3.T x), spill bf16
        ps2a = tc.alloc_tile_pool(name="ps2a", bufs=1, space="PSUM")
        ps1g = tc.alloc_tile_pool(name="ps1g", bufs=2, space="PSUM")
        ps1u = tc.alloc_tile_pool(name="ps1u", bufs=1, space="PSUM")
        w2pre = None
        w2q0 = None
        mq0 = None
        mqp = None
        # w2 panel-0 prefetch, split in chunks interleaved across the second
        # g-half so the single SWDGE queue never starves the w1/w3 stream.
        w2bounds = [0, 6, 12, 18, 24, 29, 34, 39, 44]
        w2chunks = {5 * (k + 1): k for k in range(8)}
        for gh, xTh in ((0, xT_a), (1, xT_b)):
            for fc in range(FC):
                w1t = wp.tile([P, DO, P], F32R, tag="w1")
                nc.gpsimd.dma_start(w1t, w1r[:, :, fc * P : (fc + 1) * P])
                w3t = wp.tile([P, DO, P], F32R, tag="w3")
                nc.gpsimd.dma_start(w3t, w3r[:, :, fc * P : (fc + 1) * P])
                if gh == 1 and fc in w2chunks:
                    k = w2chunks[fc]
                    lo, hi = w2bounds[k], w2bounds[k + 1]
                    nc.gpsimd.dma_start(
                        w2q0[:, lo:hi, :], w2r[:, lo:hi, 0:512]
                    )
                pg = ps1g.tile([P, 2, 512], F32, tag="pg")
                pu = ps1u.tile([P, 2, 512], F32, tag="pu")
                for d in range(DO):
                    st, sp_ = (d == 0), (d == DO - 1)
                    for j in range(2):
                        nc.tensor.matmul(
                            pg[:, j], w1t[:, d],
                            xTh[:, d, j * 512 : (j + 1) * 512],
                            start=st, stop=sp_,
                        )
                for d in range(DO):
                    st, sp_ = (d == 0), (d == DO - 1)
                    for j in range(2):
                        nc.tensor.matmul(
                            pu[:, j], w3t[:, d],
                            xTh[:, d, j * 512 : (j + 1) * 512],
                            start=st, stop=sp_,
                        )
                mo = mp.tile([P, 2, 512], BF16, tag="mo")
                nc.scalar.activation(mo, pg, mybir.ActivationFunctionType.Silu)
                nc.vector.tensor_mul(mo, mo, pu)
                nc.scalar.dma_start(
                    mid[fc * P : (fc + 1) * P, gh * GHW : (gh + 1) * GHW], mo
                )
                if gh == 1 and fc == 0:
                    # mid cols 0:1024 are complete (gh0); prefetch phase-2's
                    # first mid panel on the HWDGE queue during gh1 compute
                    mq0 = mqp.tile([P, FC, 256], BF16, tag="mq")
                    nc.sync.dma_start(mq0, mid_r[:, :, 0:256])
            if gh == 0:
                # free first-half xT; phase-2 prefetch pools take its place
                xtp_a.release()
                w2pre = tc.alloc_tile_pool(name="w2pre", bufs=1, side="right")
                w2q0 = w2pre.tile([P, FC, 512], BF16)
                mqp = tc.alloc_tile_pool(name="mqp", bufs=2, side="right")
        xtp_b.release()
        mp.release()
        wp.release()
        ps1u.release()
        ps1g.release()

        # ---- phase 2: out[g, d] = midT.T @ w2 (bf16 x bf16, fp32 psum)
        w2p = tc.alloc_tile_pool(name="w2p", bufs=2, side="right")
        op = tc.alloc_tile_pool(name="op", bufs=3, side="right")
        ps2b = tc.alloc_tile_pool(name="ps2b", bufs=3, space="PSUM")
        for dq in range(4):
            if dq == 0:
                w2q = w2q0
            else:
                w2q = w2p.tile([P, FC, 512], BF16, tag="w2q")
                nc.gpsimd.dma_start(w2q, w2r[:, :, dq * 512 : (dq + 1) * 512])
            for gp in range(8):
                if dq == 0 and gp == 0:
                    mq = mq0
                else:
                    mq = mqp.tile([P, FC, 256], BF16, tag="mq")
                    nc.sync.dma_start(mq, mid_r[:, :, gp * 256 : (gp + 1) * 256])
                if dq == 0 and gp == 0:
                    po0 = ps2a.tile([P, 512], F32, tag="pa0")
                    po1 = ps2a.tile([P, 512], F32, tag="pa1")
                else:
                    po0 = ps2b.tile([P, 512], F32, tag="po0")
                    po1 = ps2b.tile([P, 512], F32, tag="po1")
                for fo in range(FC):
                    st, sp_ = (fo == 0), (fo == FC - 1)
                    nc.tensor.matmul(
                        po0, mq[:, fo, 0:128], w2q[:, fo], start=st, stop=sp_
                    )
                    nc.tensor.matmul(
                        po1, mq[:, fo, 128:256], w2q[:, fo], start=st, stop=sp_
                    )
                for gc2, po in ((0, po0), (1, po1)):
                    ot = op.tile([P, 512], F32, tag="ot")
                    nc.vector.tensor_copy(ot, po)
                    g0 = (gp * 2 + gc2) * P
                    nc.scalar.dma_start(
                        out[g0 : g0 + P, dq * 512 : (dq + 1) * 512], ot
                    )
        op.release()
        w2p.release()
        mqp.release()
        w2pre.release()
        ps2b.release()
        ps2a.release()
        dram.release()
    nc.compile()
    return nc


_NC_CACHE = None


def _get_nc():
    global _NC_CACHE
    if _NC_CACHE is None:
        _NC_CACHE = build_nc()
    return _NC_CACHE


def _in_maps(routed_in_egD, w1, w2, w3):
    x = np.ascontiguousarray(np.asarray(routed_in_egD, dtype=np.float32))
    w1 = np.ascontiguousarray(np.asarray(w1, dtype=np.float32))
    w2 = np.ascontiguousarray(np.asarray(w2, dtype=np.float32))
    w3 = np.ascontiguousarray(np.asarray(w3, dtype=np.float32))
    x_e = x.reshape(E, G, D)
    return [
        {"x": x_e[e], "w1": w1[e], "w2": w2[e], "w3": w3[e]} for e in range(E)
    ]


def kernel(routed_in_egD, w1, w2, w3):
    nc = _get_nc()
    res = run_bass_kernel_spmd(
        nc, _in_maps(routed_in_egD, w1, w2, w3), core_ids=list(range(E))
    )
    return np.concatenate([r["out"] for r in res.results], axis=0)


def run_traced(routed_in_egD, w1, w2, w3, **trace_kwargs):
    """For test.py: run with NTFF tracing; returns (full_out, BassKernelResults)."""
    nc = _get_nc()
    res = run_bass_kernel_spmd(
        nc,
        _in_maps(routed_in_egD, w1, w2, w3),
        core_ids=list(range(E)),
        trace=True,
        **trace_kwargs,
    )
    out = np.concatenate([r["out"] for r in res.results], axis=0)
    return out, res


# revision 13
# speedup vs baseline: 1.0392x; 1.0099x over previous
"""Expert-parallel SwiGLU MLP (MoE experts) for 8 Trainium2 NeuronCores.

Problem: routed_in_egD [E*G, D] fp32, w1/w3 [E, D, F], w2 [E, F, D], E=8,
G=2048, D=2048, F=5632.  reference:
    x_egD = routed.reshape(E, G, D)
    mid   = silu(x @ w1) * (x @ w3)          # [E, G, F]
    out   = (mid @ w2).reshape(E*G, D)

Sharding: expert-parallel — core e gets expert e's x slice + weights; no
collectives.  Each core runs three 2048x2048x5632-class GEMMs (~142 GFLOP).

Per-core kernel (stage-1 matmuls fp32r, stage-2 bf16, PSUM fp32):
  phase 0: PE-transpose x [G,D] -> xT [D,G] resident in SBUF (fp32r).
  phase 1: per f-chunk (128 rows of F), per g-half: gateT/upT = w1/w3.T @ x
           accumulated over D in PSUM; SwiGLU (ACT silu -> bf16, DVE mul
           in place); midT [F,G] spilled to DRAM as bf16.
  phase 2: out[g,d] = sum_f midT[f,g]*w2[f,d]: mid panels stationary (bf16),
           w2 panels DMA-cast fp32->bf16 (moving), PSUM accumulation over F.
           Output lands in natural [G, D] layout.
"""

import numpy as np

import concourse.mybir as mybir
import concourse.tile as tile
from concourse import bacc
from concourse.bass_utils import run_bass_kernel_spmd
from concourse.masks import make_identity

E, G, D, F = 8, 2048, 2048, 5632
P = 128
DO = D // P      # 16 d-chunks
FC = F // P      # 44 f-chunks
GO = G // P      # 16 g-chunks

F32 = mybir.dt.float32
F32R = mybir.dt.float32r
BF16 = mybir.dt.bfloat16


def build_nc():
    nc = bacc.Bacc("TRN2", target_bir_lowering=False)
    x = nc.dram_tensor("x", [G, D], F32, kind="ExternalInput").ap()
    w1 = nc.dram_tensor("w1", [D, F], F32, kind="ExternalInput").ap()
    w2 = nc.dram_tensor("w2", [F, D], F32, kind="ExternalInput").ap()
    w3 = nc.dram_tensor("w3", [D, F], F32, kind="ExternalInput").ap()
    out = nc.dram_tensor("out", [G, D], F32, kind="ExternalOutput").ap()

    w1r = w1.rearrange("(do p) f -> p do f", p=P)
    w3r = w3.rearrange("(do p) f -> p do f", p=P)
    w2r = w2.rearrange("(fo p) d -> p fo d", p=P)

    with tile.TileContext(nc) as tc:
        dram = tc.alloc_tile_pool(name="dram", bufs=1, space="DRAM")
        mid = dram.tile([F, G], BF16)
        mid_r = mid.rearrange("(fo p) g -> p fo g", p=P)

        wp = tc.alloc_tile_pool(name="wp", bufs=3)
        mp = tc.alloc_tile_pool(name="mp", bufs=3)
        xtp = tc.alloc_tile_pool(name="xtp", bufs=1)
        xT = xtp.tile([P, DO, G], F32R)

        # ---- phase 0: x [G, D] -> xT [d_in, d_out, g] (fp32r)
        p0 = tc.alloc_tile_pool(name="p0", bufs=8)
        idp = tc.alloc_tile_pool(name="idp", bufs=1)
        p0ps = tc.alloc_tile_pool(name="p0ps", bufs=6, space="PSUM")
        ident = idp.tile([P, P], F32)
        make_identity(nc, ident)
        for go in range(GO):
            for q in range(4):
                xsq = p0.tile([P, 512], F32, tag="xs")
                nc.sync.dma_start(
                    xsq, x[go * P : (go + 1) * P, q * 512 : (q + 1) * 512]
                )
                tp = p0ps.tile([P, 4, P], F32, tag="tp")
                for j in range(4):
                    nc.tensor.transpose(tp[:, j], xsq[:, j * P : (j + 1) * P], ident)
                nc.vector.tensor_copy(
                    xT[:, q * 4 : (q + 1) * 4, go * P : (go + 1) * P], tp
                )
        idp.release()
        p0.release()
        p0ps.release()

        # ---- phase 1: midT[f, g] = silu(w1.T x) * (w3.T x), spill bf16
        ps1g = tc.alloc_tile_pool(name="ps1g", bufs=2, space="PSUM")
        ps1u = tc.alloc_tile_pool(name="ps1u", bufs=2, space="PSUM")
        for fc in range(FC):
            w1t = wp.tile([P, DO, P], F32R, tag="w1")
            nc.gpsimd.dma_start(w1t, w1r[:, :, fc * P : (fc + 1) * P])
            w3t = wp.tile([P, DO, P], F32R, tag="w3")
            nc.gpsimd.dma_start(w3t, w3r[:, :, fc * P : (fc + 1) * P])
            for gh in range(2):
                pg = ps1g.tile([P, 2, 512], F32, tag="pg")
                pu = ps1u.tile([P, 2, 512], F32, tag="pu")
                for d in range(DO):
                    st, sp_ = (d == 0), (d == DO - 1)
                    for j in range(2):
                        gsl = slice((gh * 2 + j) * 512, (gh * 2 + j + 1) * 512)
                        nc.tensor.matmul(
                            pg[:, j], w1t[:, d], xT[:, d, gsl], start=st, stop=sp_
                        )
                        nc.tensor.matmul(
                            pu[:, j], w3t[:, d], xT[:, d, gsl], start=st, stop=sp_
                        )
                mo = mp.tile([P, 2, 512], BF16, tag="mo")
                nc.scalar.activation(mo, pg, mybir.ActivationFunctionType.Silu)
                nc.vector.tensor_mul(mo, mo, pu)
                nc.scalar.dma_start(
                    mid[fc * P : (fc + 1) * P, gh * 1024 : (gh + 1) * 1024], mo
                )
        xtp.release()
        mp.release()
        wp.release()
        ps1u.release()
        ps1g.release()

        # ---- phase 2: out[g, d] = midT.T @ w2 (bf16 x bf16, fp32 psum)
        w2p = tc.alloc_tile_pool(name="w2p", bufs=2, side="right")
        mqp = tc.alloc_tile_pool(name="mqp", bufs=2, side="right")
        op = tc.alloc_tile_pool(name="op", bufs=4, side="right")
        ps2 = tc.alloc_tile_pool(name="ps2", bufs=2, space="PSUM")
        for dq in range(4):
            w2q = w2p.tile([P, FC, 512], BF16, tag="w2q")
            if dq == 0:
                # chunk by fo so the first accumulation steps can start early
                for k in range(4):
                    nc.gpsimd.dma_start(
                        w2q[:, k * 11 : (k + 1) * 11, :],
                        w2r[:, k * 11 : (k + 1) * 11, 0:512],
                    )
            else:
                nc.gpsimd.dma_start(w2q, w2r[:, :, dq * 512 : (dq + 1) * 512])
            for gq in range(4):
                mq = mqp.tile([P, FC, 512], BF16, tag="mq")
                if dq == 0 and gq == 0:
                    for k in range(4):
                        nc.sync.dma_start(
                            mq[:, k * 11 : (k + 1) * 11, :],
                            mid_r[:, k * 11 : (k + 1) * 11, 0:512],
                        )
                else:
                    nc.sync.dma_start(
                        mq, mid_r[:, :, gq * 512 : (gq + 1) * 512]
                    )
                po = ps2.tile([P, 4, 512], F32, tag="po")
                for fo in range(FC):
                    st, sp_ = (fo == 0), (fo == FC - 1)
                    for gc in range(4):
                        nc.tensor.matmul(
                            po[:, gc],
                            mq[:, fo, gc * P : (gc + 1) * P],
                            w2q[:, fo],
                            start=st,
                            stop=sp_,
                        )
                for gc in range(4):
                    ot = op.tile([P, 512], F32, tag="ot")
                    nc.vector.tensor_copy(ot, po[:, gc])
                    g0 = (gq * 4 + gc) * P
                    nc.scalar.dma_start(
                        out[g0 : g0 + P, dq * 512 : (dq + 1) * 512], ot
                    )
        op.release()
        mqp.release()
        w2p.release()
        ps2.release()
        dram.release()
    nc.compile()
    return nc


_NC_CACHE = None


def _get_nc():
    global _NC_CACHE
    if _NC_CACHE is None:
        _NC_CACHE = build_nc()
    return _NC_CACHE


def _in_maps(routed_in_egD, w1, w2, w3):
    x = np.ascontiguousarray(np.asarray(routed_in_egD, dtype=np.float32))
    w1 = np.ascontiguousarray(np.asarray(w1, dtype=np.float32))
    w2 = np.ascontiguousarray(np.asarray(w2, dtype=np.float32))
    w3 = np.ascontiguousarray(np.asarray(w3, dtype=np.float32))
    x_e = x.reshape(E, G, D)
    return [
        {"x": x_e[e], "w1": w1[e], "w2": w2[e], "w3": w3[e]} for e in range(E)
    ]


def kernel(routed_in_egD, w1, w2, w3):
    nc = _get_nc()
    res = run_bass_kernel_spmd(
        nc, _in_maps(routed_in_egD, w1, w2, w3), core_ids=list(range(E))
    )
    return np.concatenate([r["out"] for r in res.results], axis=0)


def run_traced(routed_in_egD, w1, w2, w3, **trace_kwargs):
    """For test.py: run with NTFF tracing; returns (full_out, BassKernelResults)."""
    nc = _get_nc()
    res = run_bass_kernel_spmd(
        nc,
        _in_maps(routed_in_egD, w1, w2, w3),
        core_ids=list(range(E)),
        trace=True,
        **trace_kwargs,
    )
    out = np.concatenate([r["out"] for r in res.results], axis=0)
    return out, res


# revision 14
# speedup vs baseline: 1.0707x; 1.0302x over previous
"""Expert-parallel SwiGLU MLP (MoE experts) for 8 Trainium2 NeuronCores.

Problem: routed_in_egD [E*G, D] fp32, w1/w3 [E, D, F], w2 [E, F, D], E=8,
G=2048, D=2048, F=5632.  reference:
    x_egD = routed.reshape(E, G, D)
    mid   = silu(x @ w1) * (x @ w3)          # [E, G, F]
    out   = (mid @ w2).reshape(E*G, D)

Sharding: expert-parallel — core e gets expert e's x slice + weights; no
collectives.  Each core runs three 2048x2048x5632-class GEMMs (~142 GFLOP).

Per-core kernel (stage-1 matmuls fp32r, stage-2 bf16, PSUM fp32):
  phase 0: PE-transpose x [G,D] -> xT [D,G] resident in SBUF (fp32r).
  phase 1: per f-chunk (128 rows of F), per g-half: gateT/upT = w1/w3.T @ x
           accumulated over D in PSUM; SwiGLU (ACT silu -> bf16, DVE mul
           in place); midT [F,G] spilled to DRAM as bf16.
  phase 2: out[g,d] = sum_f midT[f,g]*w2[f,d]: mid panels stationary (bf16),
           w2 panels DMA-cast fp32->bf16 (moving), PSUM accumulation over F.
           Output lands in natural [G, D] layout.
"""

import numpy as np

import concourse.mybir as mybir
import concourse.tile as tile
from concourse import bacc
from concourse.bass_utils import run_bass_kernel_spmd
from concourse.masks import make_identity

E, G, D, F = 8, 2048, 2048, 5632
P = 128
DO = D // P      # 16 d-chunks
FC = F // P      # 44 f-chunks
GO = G // P      # 16 g-chunks

F32 = mybir.dt.float32
F32R = mybir.dt.float32r
BF16 = mybir.dt.bfloat16


def build_nc():
    nc = bacc.Bacc("TRN2", target_bir_lowering=False)
    x = nc.dram_tensor("x", [G, D], F32, kind="ExternalInput").ap()
    w1 = nc.dram_tensor("w1", [D, F], F32, kind="ExternalInput").ap()
    w2 = nc.dram_tensor("w2", [F, D], F32, kind="ExternalInput").ap()
    w3 = nc.dram_tensor("w3", [D, F], F32, kind="ExternalInput").ap()
    out = nc.dram_tensor("out", [G, D], F32, kind="ExternalOutput").ap()

    w1r = w1.rearrange("(do p) f -> p do f", p=P)
    w3r = w3.rearrange("(do p) f -> p do f", p=P)
    w2r = w2.rearrange("(fo p) d -> p fo d", p=P)

    with tile.TileContext(nc) as tc:
        dram = tc.alloc_tile_pool(name="dram", bufs=1, space="DRAM")
        # midT stored gq-blocked and f-major within each g-panel:
        # mid5[p, gq, fo, g'] = silu/up product for f = fo*128+p, g = gq*512+g'.
        # Phase-1 writes are per-partition contiguous 1KB; phase-2 panel reads
        # are per-partition contiguous 44KB (128 DMA blocks instead of 5632).
        mid5 = dram.tile([P, 4, FC, 512], BF16)

        wp = tc.alloc_tile_pool(name="wp", bufs=3)
        mp = tc.alloc_tile_pool(name="mp", bufs=3)
        xtp = tc.alloc_tile_pool(name="xtp", bufs=1)
        xT = xtp.tile([P, DO, G], F32R)

        # ---- phase 0: x [G, D] -> xT [d_in, d_out, g] (fp32r)
        p0 = tc.alloc_tile_pool(name="p0", bufs=8)
        idp = tc.alloc_tile_pool(name="idp", bufs=1)
        p0ps = tc.alloc_tile_pool(name="p0ps", bufs=6, space="PSUM")
        ident = idp.tile([P, P], F32)
        make_identity(nc, ident)
        for go in range(GO):
            for q in range(4):
                xsq = p0.tile([P, 512], F32, tag="xs")
                nc.sync.dma_start(
                    xsq, x[go * P : (go + 1) * P, q * 512 : (q + 1) * 512]
                )
                tp = p0ps.tile([P, 4, P], F32, tag="tp")
                for j in range(4):
                    nc.tensor.transpose(tp[:, j], xsq[:, j * P : (j + 1) * P], ident)
                nc.vector.tensor_copy(
                    xT[:, q * 4 : (q + 1) * 4, go * P : (go + 1) * P], tp
                )
        idp.release()
        p0.release()
        p0ps.release()

        # ---- phase 1: midT[f, g] = silu(w1.T x) * (w3.T x), spill bf16
        ps1g = tc.alloc_tile_pool(name="ps1g", bufs=2, space="PSUM")
        ps1u = tc.alloc_tile_pool(name="ps1u", bufs=2, space="PSUM")
        for fc in range(FC):
            w1t = wp.tile([P, DO, P], F32R, tag="w1")
            nc.gpsimd.dma_start(w1t, w1r[:, :, fc * P : (fc + 1) * P])
            w3t = wp.tile([P, DO, P], F32R, tag="w3")
            nc.gpsimd.dma_start(w3t, w3r[:, :, fc * P : (fc + 1) * P])
            for gh in range(2):
                pg = ps1g.tile([P, 2, 512], F32, tag="pg")
                pu = ps1u.tile([P, 2, 512], F32, tag="pu")
                for d in range(DO):
                    st, sp_ = (d == 0), (d == DO - 1)
                    for j in range(2):
                        gsl = slice((gh * 2 + j) * 512, (gh * 2 + j + 1) * 512)
                        nc.tensor.matmul(
                            pg[:, j], w1t[:, d], xT[:, d, gsl], start=st, stop=sp_
                        )
                        nc.tensor.matmul(
                            pu[:, j], w3t[:, d], xT[:, d, gsl], start=st, stop=sp_
                        )
                mo = mp.tile([P, 2, 512], BF16, tag="mo")
                nc.scalar.activation(mo, pg, mybir.ActivationFunctionType.Silu)
                nc.vector.tensor_mul(mo, mo, pu)
                for j in range(2):
                    nc.scalar.dma_start(mid5[:, gh * 2 + j, fc], mo[:, j])
        xtp.release()
        mp.release()
        wp.release()
        ps1u.release()
        ps1g.release()

        # ---- phase 2: out[g, d] = midT.T @ w2 (bf16 x bf16, fp32 psum)
        w2p = tc.alloc_tile_pool(name="w2p", bufs=2, side="right")
        mqp = tc.alloc_tile_pool(name="mqp", bufs=2, side="right")
        op = tc.alloc_tile_pool(name="op", bufs=4, side="right")
        ps2 = tc.alloc_tile_pool(name="ps2", bufs=2, space="PSUM")
        for dq in range(4):
            w2q = w2p.tile([P, FC, 512], BF16, tag="w2q")
            if dq == 0:
                # chunk by fo so the first accumulation steps can start early
                for k in range(4):
                    nc.gpsimd.dma_start(
                        w2q[:, k * 11 : (k + 1) * 11, :],
                        w2r[:, k * 11 : (k + 1) * 11, 0:512],
                    )
            else:
                nc.gpsimd.dma_start(w2q, w2r[:, :, dq * 512 : (dq + 1) * 512])
            for gq in range(4):
                mq = mqp.tile([P, FC, 512], BF16, tag="mq")
                if dq == 0 and gq == 0:
                    for k in range(4):
                        nc.sync.dma_start(
                            mq[:, k * 11 : (k + 1) * 11, :],
                            mid5[:, 0, k * 11 : (k + 1) * 11, :],
                        )
                else:
                    nc.sync.dma_start(mq, mid5[:, gq])
                po = ps2.tile([P, 4, 512], F32, tag="po")
                for fo in range(FC):
                    st, sp_ = (fo == 0), (fo == FC - 1)
                    for gc in range(4):
                        nc.tensor.matmul(
                            po[:, gc],
                            mq[:, fo, gc * P : (gc + 1) * P],
                            w2q[:, fo],
                            start=st,
                            stop=sp_,
                        )
                for gc in range(4):
                    ot = op.tile([P, 512], F32, tag="ot")
                    nc.vector.tensor_copy(ot, po[:, gc])
                    g0 = (gq * 4 + gc) * P
                    nc.scalar.dma_start(
                        out[g0 : g0 + P, dq * 512 : (dq + 1) * 512], ot
                    )
        op.release()
        mqp.release()
        w2p.release()
        ps2.release()
        dram.release()
    nc.compile()
    return nc


_NC_CACHE = None


def _get_nc():
    global _NC_CACHE
    if _NC_CACHE is None:
        _NC_CACHE = build_nc()
    return _NC_CACHE


def _in_maps(routed_in_egD, w1, w2, w3):
    x = np.ascontiguousarray(np.asarray(routed_in_egD, dtype=np.float32))
    w1 = np.ascontiguousarray(np.asarray(w1, dtype=np.float32))
    w2 = np.ascontiguousarray(np.asarray(w2, dtype=np.float32))
    w3 = np.ascontiguousarray(np.asarray(w3, dtype=np.float32))
    x_e = x.reshape(E, G, D)
    return [
        {"x": x_e[e], "w1": w1[e], "w2": w2[e], "w3": w3[e]} for e in range(E)
    ]


def kernel(routed_in_egD, w1, w2, w3):
    nc = _get_nc()
    res = run_bass_kernel_spmd(
        nc, _in_maps(routed_in_egD, w1, w2, w3), core_ids=list(range(E))
    )
    return np.concatenate([r["out"] for r in res.results], axis=0)


def run_traced(routed_in_egD, w1, w2, w3, **trace_kwargs):
    """For test.py: run with NTFF tracing; returns (full_out, BassKernelResults)."""
    nc = _get_nc()
    res = run_bass_kernel_spmd(
        nc,
        _in_maps(routed_in_egD, w1, w2, w3),
        core_ids=list(range(E)),
        trace=True,
        **trace_kwargs,
    )
    out = np.concatenate([r["out"] for r in res.results], axis=0)
    return out, res


# revision 15
# speedup vs baseline: 1.0717x; 1.0010x over previous
"""Expert-parallel SwiGLU MLP (MoE experts) for 8 Trainium2 NeuronCores.

Problem: routed_in_egD [E*G, D] fp32, w1/w3 [E, D, F], w2 [E, F, D], E=8,
G=2048, D=2048, F=5632.  reference:
    x_egD = routed.reshape(E, G, D)
    mid   = silu(x @ w1) * (x @ w3)          # [E, G, F]
    out   = (mid @ w2).reshape(E*G, D)

Sharding: expert-parallel — core e gets expert e's x slice + weights; no
collectives.  Each core runs three 2048x2048x5632-class GEMMs (~142 GFLOP).

Per-core kernel (stage-1 matmuls fp32r, stage-2 bf16, PSUM fp32):
  phase 0: PE-transpose x [G,D] -> xT [D,G] resident in SBUF (fp32r).
  phase 1: per f-chunk (128 rows of F), per g-half: gateT/upT = w1/w3.T @ x
           accumulated over D in PSUM; SwiGLU (ACT silu -> bf16, DVE mul
           in place); midT [F,G] spilled to DRAM as bf16.
  phase 2: out[g,d] = sum_f midT[f,g]*w2[f,d]: mid panels stationary (bf16),
           w2 panels DMA-cast fp32->bf16 (moving), PSUM accumulation over F.
           Output lands in natural [G, D] layout.
"""

import numpy as np

import concourse.mybir as mybir
import concourse.tile as tile
from concourse import bacc
from concourse.bass_utils import run_bass_kernel_spmd
from concourse.masks import make_identity

E, G, D, F = 8, 2048, 2048, 5632
P = 128
DO = D // P      # 16 d-chunks
FC = F // P      # 44 f-chunks
GO = G // P      # 16 g-chunks

F32 = mybir.dt.float32
F32R = mybir.dt.float32r
BF16 = mybir.dt.bfloat16


def build_nc():
    nc = bacc.Bacc("TRN2", target_bir_lowering=False)
    x = nc.dram_tensor("x", [G, D], F32, kind="ExternalInput").ap()
    w1 = nc.dram_tensor("w1", [D, F], F32, kind="ExternalInput").ap()
    w2 = nc.dram_tensor("w2", [F, D], F32, kind="ExternalInput").ap()
    w3 = nc.dram_tensor("w3", [D, F], F32, kind="ExternalInput").ap()
    out = nc.dram_tensor("out", [G, D], F32, kind="ExternalOutput").ap()

    w1r = w1.rearrange("(do p) f -> p do f", p=P)
    w3r = w3.rearrange("(do p) f -> p do f", p=P)
    w2r = w2.rearrange("(fo p) d -> p fo d", p=P)

    with tile.TileContext(nc) as tc:
        dram = tc.alloc_tile_pool(name="dram", bufs=1, space="DRAM")
        # midT stored gp-blocked and f-major within each g-panel:
        # mid5[p, gp, fo, g'] = silu/up product for f = fo*128+p, g = gp*256+g'.
        # Phase-1 writes are per-partition contiguous 512B; phase-2 panel reads
        # are per-partition contiguous 22KB (128 DMA blocks instead of 5632).
        mid5 = dram.tile([P, 8, FC, 256], BF16)

        wp = tc.alloc_tile_pool(name="wp", bufs=3)
        mp = tc.alloc_tile_pool(name="mp", bufs=3)
        xtp = tc.alloc_tile_pool(name="xtp", bufs=1)
        xT = xtp.tile([P, DO, G], F32R)

        # ---- phase 0: x [G, D] -> xT [d_in, d_out, g] (fp32r)
        p0 = tc.alloc_tile_pool(name="p0", bufs=8)
        idp = tc.alloc_tile_pool(name="idp", bufs=1)
        p0ps = tc.alloc_tile_pool(name="p0ps", bufs=6, space="PSUM")
        ident = idp.tile([P, P], F32)
        make_identity(nc, ident)
        for go in range(GO):
            for q in range(4):
                xsq = p0.tile([P, 512], F32, tag="xs")
                nc.sync.dma_start(
                    xsq, x[go * P : (go + 1) * P, q * 512 : (q + 1) * 512]
                )
                tp = p0ps.tile([P, 4, P], F32, tag="tp")
                for j in range(4):
                    nc.tensor.transpose(tp[:, j], xsq[:, j * P : (j + 1) * P], ident)
                nc.vector.tensor_copy(
                    xT[:, q * 4 : (q + 1) * 4, go * P : (go + 1) * P], tp
                )
        idp.release()
        p0.release()
        p0ps.release()

        # ---- phase 1: midT[f, g] = silu(w1.T x) * (w3.T x), spill bf16
        ps1g = tc.alloc_tile_pool(name="ps1g", bufs=2, space="PSUM")
        ps1u = tc.alloc_tile_pool(name="ps1u", bufs=2, space="PSUM")
        for fc in range(FC):
            w1t = wp.tile([P, DO, P], F32R, tag="w1")
            nc.gpsimd.dma_start(w1t, w1r[:, :, fc * P : (fc + 1) * P])
            w3t = wp.tile([P, DO, P], F32R, tag="w3")
            nc.gpsimd.dma_start(w3t, w3r[:, :, fc * P : (fc + 1) * P])
            for gh in range(2):
                pg = ps1g.tile([P, 2, 512], F32, tag="pg")
                pu = ps1u.tile([P, 2, 512], F32, tag="pu")
                for d in range(DO):
                    st, sp_ = (d == 0), (d == DO - 1)
                    for j in range(2):
                        gsl = slice((gh * 2 + j) * 512, (gh * 2 + j + 1) * 512)
                        nc.tensor.matmul(
                            pg[:, j], w1t[:, d], xT[:, d, gsl], start=st, stop=sp_
                        )
                        nc.tensor.matmul(
                            pu[:, j], w3t[:, d], xT[:, d, gsl], start=st, stop=sp_
                        )
                mo = mp.tile([P, 4, 256], BF16, tag="mo")
                nc.scalar.activation(
                    mo, pg.rearrange("p j g -> p (j g)"),
                    mybir.ActivationFunctionType.Silu,
                )
                nc.vector.tensor_mul(mo, mo, pu.rearrange("p j g -> p (j g)"))
                for j in range(4):
                    nc.scalar.dma_start(mid5[:, gh * 4 + j, fc], mo[:, j])
        xtp.release()
        mp.release()
        wp.release()
        ps1u.release()
        ps1g.release()

        # ---- phase 2: out[g, d] = midT.T @ w2 (bf16 x bf16, fp32 psum)
        w2p = tc.alloc_tile_pool(name="w2p", bufs=2, side="right")
        mqp = tc.alloc_tile_pool(name="mqp", bufs=3, side="right")
        op = tc.alloc_tile_pool(name="op", bufs=6, side="right")
        ps2 = tc.alloc_tile_pool(name="ps2", bufs=3, space="PSUM")
        w2bounds = [0, 6, 12, 18, 24, 29, 34, 39, 44]
        for dq in range(4):
            w2q = w2p.tile([P, FC, 512], BF16, tag="w2q")
            if dq == 0:
                # chunk by fo so the first accumulation steps can start early
                for k in range(8):
                    lo, hi = w2bounds[k], w2bounds[k + 1]
                    nc.gpsimd.dma_start(
                        w2q[:, lo:hi, :], w2r[:, lo:hi, 0:512]
                    )
            else:
                nc.gpsimd.dma_start(w2q, w2r[:, :, dq * 512 : (dq + 1) * 512])
            for gp in range(8):
                mq = mqp.tile([P, FC, 256], BF16, tag="mq")
                # alternate HWDGE queues so consecutive panel loads overlap
                dma_eng = nc.sync if gp % 2 == 0 else nc.scalar
                if dq == 0 and gp == 0:
                    for k in range(8):
                        lo, hi = w2bounds[k], w2bounds[k + 1]
                        nc.sync.dma_start(
                            mq[:, lo:hi, :], mid5[:, 0, lo:hi, :]
                        )
                else:
                    dma_eng.dma_start(mq, mid5[:, gp])
                po = ps2.tile([P, 2, 512], F32, tag="po")
                for fo in range(FC):
                    st, sp_ = (fo == 0), (fo == FC - 1)
                    for gc in range(2):
                        nc.tensor.matmul(
                            po[:, gc],
                            mq[:, fo, gc * P : (gc + 1) * P],
                            w2q[:, fo],
                            start=st,
                            stop=sp_,
                        )
                for gc in range(2):
                    ot = op.tile([P, 512], F32, tag="ot")
                    nc.vector.tensor_copy(ot, po[:, gc])
                    g0 = (gp * 2 + gc) * P
                    nc.scalar.dma_start(
                        out[g0 : g0 + P, dq * 512 : (dq + 1) * 512], ot
                    )
        op.release()
        mqp.release()
        w2p.release()
        ps2.release()
        dram.release()
    nc.compile()
    return nc


_NC_CACHE = None


def _get_nc():
    global _NC_CACHE
    if _NC_CACHE is None:
        _NC_CACHE = build_nc()
    return _NC_CACHE


def _in_maps(routed_in_egD, w1, w2, w3):
    x = np.ascontiguousarray(np.asarray(routed_in_egD, dtype=np.float32))
    w1 = np.ascontiguousarray(np.asarray(w1, dtype=np.float32))
    w2 = np.ascontiguousarray(np.asarray(w2, dtype=np.float32))
    w3 = np.ascontiguousarray(np.asarray(w3, dtype=np.float32))
    x_e = x.reshape(E, G, D)
    return [
        {"x": x_e[e], "w1": w1[e], "w2": w2[e], "w3": w3[e]} for e in range(E)
    ]


def kernel(routed_in_egD, w1, w2, w3):
    nc = _get_nc()
    res = run_bass_kernel_spmd(
        nc, _in_maps(routed_in_egD, w1, w2, w3), core_ids=list(range(E))
    )
    return np.concatenate([r["out"] for r in res.results], axis=0)


def run_traced(routed_in_egD, w1, w2, w3, **trace_kwargs):
    """For test.py: run with NTFF tracing; returns (full_out, BassKernelResults)."""
    nc = _get_nc()
    res = run_bass_kernel_spmd(
        nc,
        _in_maps(routed_in_egD, w1, w2, w3),
        core_ids=list(range(E)),
        trace=True,
        **trace_kwargs,
    )
    out = np.concatenate([r["out"] for r in res.results], axis=0)
    return out, res
